# revision 22
# baseline (speedup 1.0000x reference)
"""Trainium2 Bass kernel for nn_AttentionBlock (column-softmax causal attention).

Reference computation (B=4, S=4096, D=128, K=64, V=128):
    Q = x @ Wq.T + bq            [B,S,64]
    Km = x @ Wk.T + bk           [B,S,64]
    Vm = x @ Wv.T + bv           [B,S,128]
    s  = Q @ Km.T / 8            [B,S,S], causal mask j>q -> -1e9
    p  = softmax(s, axis=1)      (softmax over the QUERY axis -- column softmax)
    att = p @ Vm                 [B,S,128]
    out = concat(x, att, dim=2)  [B,S,256]

Key observation: with ST = s.T (layout [j, q]) the softmax denominator
l[j] = sum_q exp(ST[j, q]) is a free-dim reduction, so
att[q] = sum_j exp(ST[j,q]) * (Vm[j]/l[j]) -- a flash-style two-phase kernel
with NO max subtraction needed (scores are O(+-20), exp is safe in fp32).

Sharding (8 cores): core c -> batch b = c//2, j-tile parity p = c%2.
Each core computes l[j] and the PV partial sum for its 16 j-tiles
(j-tile J = 2*i + p), over all q. Host adds the two partials per batch.
All parity differences are data-driven (xkv row gather + additive mask
input), so one SPMD program serves all 8 cores.

Performance structure per core:
  - QK score matmuls run as f32r with row-PAIR packing: rows 2r / 2r+1 use
    the two 64-partition halves of the PE array concurrently (KT/QT are
    duplicated into both partition halves so tile_position auto-derives).
  - exp runs on ACT with fused per-partition accumulation (accum_out = l).
  - PV runs transposed: attT[v, q] = sum_j Vp[j,v] * E[j,q] with N=512
    moving operands (bf16), then PE-transposes back to [q, v] tiles.
"""

import numpy as np

B, S, D = 4, 4096, 128
KD, VD = 64, 128
P = 128
NCORES = 8
JT = 16           # local j-tiles per core
NT = S // P       # 32 global q/j tiles
CHUNK = 1536      # ACT exp chunk width (PSUM cols)

QK_F32R = True

ROW_W = [S - 2 * i * P for i in range(JT)]          # E row widths
EOFF = [0] * JT
for _i in range(1, JT):
    EOFF[_i] = EOFF[_i - 1] + ROW_W[_i - 1]
ECOLS = EOFF[-1] + ROW_W[-1]                        # 34816

_CACHE = {}


def _build_program():
    from contextlib import ExitStack

    from concourse import bacc, mybir
    from concourse import tile as tile_mod

    dt = mybir.dt
    f32, bf16 = dt.float32, dt.bfloat16
    Alu = mybir.AluOpType
    ActF = mybir.ActivationFunctionType

    nc = bacc.Bacc(
        "TRN2", target_bir_lowering=False, debug=False, num_devices=NCORES
    )

    # Operand tiles of f32r matmuls must be PRODUCED as float32r (the BIR
    # verifier requires the producing instruction to round); they are
    # written by DVE ops (which round) or DMA'd in as float32r directly.
    mmdt = dt.float32r if QK_F32R else f32

    # Host supplies x^T / xkv^T / W^T (pure layout prep) so the kernel
    # spends no PE/ACT/DVE time transposing, and the DMAs are contiguous.
    xt_d = nc.dram_tensor("xt", [P, S], mmdt, kind="ExternalInput").ap()
    xkvt_d = nc.dram_tensor("xkvt", [P, JT * P], mmdt, kind="ExternalInput").ap()
    wqt_d = nc.dram_tensor("wqt", [P, 2 * KD], mmdt, kind="ExternalInput").ap()
    bq_d = nc.dram_tensor("bq", [P, 1], f32, kind="ExternalInput").ap()
    wkt_d = nc.dram_tensor("wkt", [P, 2 * KD], mmdt, kind="ExternalInput").ap()
    bk_d = nc.dram_tensor("bk", [P, 1], f32, kind="ExternalInput").ap()
    wvt_d = nc.dram_tensor("wvt", [P, VD], mmdt, kind="ExternalInput").ap()
    bv_d = nc.dram_tensor("bv", [VD, 1], f32, kind="ExternalInput").ap()
    mrow_d = nc.dram_tensor("mrow", [P, 2 * P], f32, kind="ExternalInput").ap()
    att_d = nc.dram_tensor("att", [S, VD], f32, kind="ExternalOutput").ap()

    with tile_mod.TileContext(nc) as tc, ExitStack() as ctx:
        persist = ctx.enter_context(tc.tile_pool(name="persist", bufs=1))

        xT = persist.tile([P, S], mmdt)            # [d, q]
        xkvT = persist.tile([P, JT * P], mmdt)     # [d, local j]
        # QT/KT live in BOTH partition halves (rows 0-63 == rows 64-127) so
        # QK row pairs can use tile_position (0,0)/(64,0) concurrently.
        QT = persist.tile([P, S], mmdt)            # [k(dup), q]
        KTl = persist.tile([P, JT * P], mmdt)      # [k(dup), local j]
        V_sb = persist.tile([P, JT, VD], f32)      # [local j, v]
        Vp_sb = persist.tile([P, JT, VD], bf16)    # V / l
        E_all = persist.tile([P, ECOLS], bf16)     # exp(scores.T) rows
        l_all = persist.tile([P, JT], f32)
        linv = persist.tile([P, JT], f32)
        WqT = persist.tile([P, 2 * KD], mmdt)      # Wq^T duplicated in free dim
        WkT = persist.tile([P, 2 * KD], mmdt)
        WvT = persist.tile([P, VD], mmdt)
        VT_sb = persist.tile([P, JT * P], f32)     # [v, local j]
        bq_sb = persist.tile([P, 1], f32)
        bk_sb = persist.tile([P, 1], f32)
        bv_sb = persist.tile([VD, 1], f32)
        mrow_sb = persist.tile([P, 2 * P], f32)
        ident = persist.tile([P, P], f32)

        # ---- input DMAs: tiny tensors first, then contiguous transposed
        # activations chunked so dependent compute starts early.
        nc.sync.dma_start(out=WqT, in_=wqt_d)
        nc.sync.dma_start(out=WkT, in_=wkt_d)
        nc.sync.dma_start(out=WvT, in_=wvt_d)
        nc.sync.dma_start(out=bq_sb, in_=bq_d)
        nc.sync.dma_start(out=bk_sb, in_=bk_d)
        nc.sync.dma_start(out=bv_sb, in_=bv_d)
        nc.sync.dma_start(out=mrow_sb, in_=mrow_d)
        for g in range(4):
            nc.sync.dma_start(
                out=xkvT[:, g * 512 : (g + 1) * 512],
                in_=xkvt_d[:, g * 512 : (g + 1) * 512],
            )
        for g in reversed(range(8)):
            nc.sync.dma_start(
                out=xT[:, g * 512 : (g + 1) * 512],
                in_=xt_d[:, g * 512 : (g + 1) * 512],
            )

        # identity for PE transposes
        nc.gpsimd.memset(ident, 0.0)
        nc.gpsimd.affine_select(
            out=ident,
            in_=ident,
            compare_op=Alu.not_equal,
            fill=1.0,
            base=0,
            pattern=[[-1, P]],
            channel_multiplier=1,
        )

        # ---- phase A/B: projections, then row pairs in REVERSE order ---
        # Pair 7 depends only on the tail of QT, so exp work starts a few
        # microseconds in; the ACT engine (the phase bottleneck) never waits
        # for the full projection sweep.
        with ExitStack() as pha:
            prj = pha.enter_context(
                tc.tile_pool(name="prj_psum", bufs=2, space="PSUM")
            )
            rowp = pha.enter_context(
                tc.tile_pool(name="row_psum", bufs=2, space="PSUM")
            )
            lpp = pha.enter_context(tc.tile_pool(name="lparts", bufs=8))

            # KT local = Wk @ xkv^T (+bk), duplicated into both halves
            for c in reversed(range(JT * P // 512)):
                ps = prj.tile([P, 512], f32, tag="prj", name=f"kt_{c}")
                nc.tensor.matmul(
                    ps,
                    lhsT=WkT,
                    rhs=xkvT[:, c * 512 : (c + 1) * 512],
                    start=True,
                    stop=True,
                )
                nc.vector.tensor_scalar(
                    out=KTl[:, c * 512 : (c + 1) * 512],
                    in0=ps,
                    scalar1=bk_sb,
                    scalar2=None,
                    op0=Alu.add,
                )
            # QT = Wq_s @ x^T (+bq), duplicated into both halves
            for c in reversed(range(S // 512)):
                ps = prj.tile([P, 512], f32, tag="prj", name=f"qt_{c}")
                nc.tensor.matmul(
                    ps,
                    lhsT=WqT,
                    rhs=xT[:, c * 512 : (c + 1) * 512],
                    start=True,
                    stop=True,
                )
                nc.vector.tensor_scalar(
                    out=QT[:, c * 512 : (c + 1) * 512],
                    in0=ps,
                    scalar1=bq_sb,
                    scalar2=None,
                    op0=Alu.add,
                )

            def emit_row_pair(r):
                # rows 2r (partition half 0) and 2r+1 (half 64), MMs
                # interleaved at 512-slice granularity so the PE overlaps
                # them in opposite array halves.
                state = {}
                for i in (2 * r, 2 * r + 1):
                    q0 = 256 * i
                    w = ROW_W[i]
                    chunks = [
                        (q0 + c * CHUNK, min(CHUNK, w - c * CHUNK))
                        for c in range((w + CHUNK - 1) // CHUNK)
                    ]
                    slices = []
                    for ci, (off, cw) in enumerate(chunks):
                        for s0 in range(0, cw, 512):
                            slices.append((ci, off, cw, s0, min(512, cw - s0)))
                    state[i] = {"chunks": chunks, "slices": slices, "ps": {}}

                def finish_chunk(i, ci, cw):
                    st = state[i]
                    ps = st["ps"][ci]
                    if ci == 0:
                        nc.vector.tensor_add(
                            ps[:, : 2 * P], ps[:, : 2 * P], mrow_sb
                        )
                    lp = lpp.tile([P, 1], f32, tag="lp", name=f"lp_{i}_{ci}")
                    ecol = EOFF[i] + ci * CHUNK
                    nc.scalar.activation(
                        out=E_all[:, ecol : ecol + cw],
                        in_=ps[:, :cw],
                        func=ActF.Exp,
                        accum_out=lp,
                    )
                    if ci == 0:
                        nc.vector.tensor_copy(l_all[:, i : i + 1], lp)
                    else:
                        nc.vector.tensor_add(
                            l_all[:, i : i + 1], l_all[:, i : i + 1], lp
                        )

                nslice = max(len(state[i]["slices"]) for i in state)
                for k in range(nslice):
                    for idx, i in enumerate((2 * r, 2 * r + 1)):
                        st = state[i]
                        if k >= len(st["slices"]):
                            continue
                        ci, off, cw, s0, sw = st["slices"][k]
                        if ci not in st["ps"]:
                            st["ps"][ci] = rowp.tile(
                                [P, CHUNK], f32, tag="st", name=f"st_{i}_{ci}"
                            )
                        base = KD * idx
                        nc.tensor.matmul(
                            st["ps"][ci][:, s0 : s0 + sw],
                            lhsT=KTl[base : base + KD, i * P : (i + 1) * P],
                            rhs=QT[base : base + KD, off + s0 : off + s0 + sw],
                            start=True,
                            stop=True,
                        )
                        if s0 + sw == cw:
                            finish_chunk(i, ci, cw)

            for r in reversed(range(8)):
                emit_row_pair(r)

            # VT = Wv @ xkv^T (+bv) — emitted last so it fills the PE while
            # the final exps drain on ACT.
            for c in range(JT * P // 512):
                ps = prj.tile([P, 512], f32, tag="prj", name=f"vt_{c}")
                nc.tensor.matmul(
                    ps,
                    lhsT=WvT,
                    rhs=xkvT[:, c * 512 : (c + 1) * 512],
                    start=True,
                    stop=True,
                )
                nc.vector.tensor_scalar(
                    out=VT_sb[:, c * 512 : (c + 1) * 512],
                    in0=ps,
                    scalar1=bv_sb,
                    scalar2=None,
                    op0=Alu.add,
                )

        # ---- phase C: V transposes, V' = V/l, PV block ------------------
        with ExitStack() as phc:
            attp = phc.enter_context(
                tc.tile_pool(name="att_psum", bufs=4, space="PSUM")
            )
            tsbp = phc.enter_context(tc.tile_pool(name="attT_sb", bufs=2))
            sbo = phc.enter_context(tc.tile_pool(name="att_sb", bufs=2))

            for grp in range(JT // 4):
                ps = attp.tile([P, 4, P], f32, tag="attT", name=f"vtp_{grp}")
                for k in range(4):
                    i = grp * 4 + k
                    nc.tensor.transpose(
                        ps[:, k, :], VT_sb[:, i * P : (i + 1) * P], ident
                    )
                nc.vector.tensor_copy(
                    V_sb[:, grp * 4 : (grp + 1) * 4, :].rearrange(
                        "p a b -> p (a b)"
                    ),
                    ps.rearrange("p a b -> p (a b)"),
                )
            nc.vector.reciprocal(linv, l_all)
            for i in range(JT):
                nc.vector.tensor_scalar(
                    out=Vp_sb[:, i, :],
                    in0=V_sb[:, i, :],
                    scalar1=linv[:, i : i + 1],
                    scalar2=None,
                    op0=Alu.mult,
                )

            def emit_pv_chunk(c):
                # attT[v, q] for q in [512c, 512c+512): rows i <= 2c full
                # coverage (N=512), row 2c+1 covers the second half (N=256).
                attT = attp.tile([P, 4, P], f32, tag="attT", name=f"attT_{c}")
                aflat = attT.rearrange("p a b -> p (a b)")
                for i in range(2 * c + 1):
                    ecol = EOFF[i] + 512 * c - 256 * i
                    nc.tensor.matmul(
                        aflat,
                        lhsT=Vp_sb[:, i, :],
                        rhs=E_all[:, ecol : ecol + 512],
                        start=(i == 0),
                        stop=False,
                    )
                i2 = 2 * c + 1
                nc.tensor.matmul(
                    aflat[:, 256:512],
                    lhsT=Vp_sb[:, i2, :],
                    rhs=E_all[:, EOFF[i2] : EOFF[i2] + 256],
                    start=False,
                    stop=True,
                )
                tsb = tsbp.tile([P, 4, P], f32, tag="tsb", name=f"tsb_{c}")
                nc.scalar.copy(tsb.rearrange("p a b -> p (a b)"), aflat)
                outq = attp.tile([P, 4, P], f32, tag="attT", name=f"outq_{c}")
                for k in range(4):
                    nc.tensor.transpose(outq[:, k, :], tsb[:, k, :], ident)
                osb = sbo.tile([P, 4, P], f32, tag="osb", name=f"osb_{c}")
                nc.vector.tensor_copy(
                    osb.rearrange("p a b -> p (a b)"),
                    outq.rearrange("p a b -> p (a b)"),
                )
                nc.sync.dma_start(
                    out=att_d[c * 512 : (c + 1) * 512, :].rearrange(
                        "(t p) v -> p t v", p=P
                    ),
                    in_=osb,
                )

            for c in reversed(range(8)):
                emit_pv_chunk(c)

    nc.compile()
    return nc


def _host_inputs(x, Wq, bq, Wk, bk, Wv, bv):
    """Per-core input maps (host does layout prep: transposes + gathers)."""
    x_full = np.ascontiguousarray(x, dtype=np.float32)
    Wq_s = np.asarray(Wq, np.float32) / 8.0
    wqt = np.ascontiguousarray(np.concatenate([Wq_s.T, Wq_s.T], axis=1))
    bq_s = np.tile((np.asarray(bq, np.float32) / 8.0).reshape(KD, 1), (2, 1))
    WkT_ = np.asarray(Wk, np.float32).T
    wkt = np.ascontiguousarray(np.concatenate([WkT_, WkT_], axis=1))
    bk_ = np.tile(np.asarray(bk, np.float32).reshape(KD, 1), (2, 1))
    wvt = np.ascontiguousarray(np.asarray(Wv, np.float32).T)
    bv_ = np.asarray(bv, np.float32).reshape(VD, 1)

    tri = np.where(
        np.arange(P)[None, :] >= np.arange(P)[:, None], 0.0, -1e9
    ).astype(np.float32)
    mrows = []
    for p in (0, 1):
        m = np.zeros((P, 2 * P), np.float32)
        if p == 0:
            m[:, :P] = tri
        else:
            m[:, :P] = -1e9
            m[:, P:] = tri
        mrows.append(m)

    in_maps = []
    xts = [np.ascontiguousarray(x_full[b].T) for b in range(B)]
    for c in range(NCORES):
        b, p = c // 2, c % 2
        xkvt = np.ascontiguousarray(
            x_full[b].reshape(NT, P, D)[p::2].reshape(JT * P, D).T
        )
        in_maps.append(
            {
                "xt": xts[b],
                "xkvt": xkvt,
                "wqt": wqt,
                "bq": bq_s,
                "wkt": wkt,
                "bk": bk_,
                "wvt": wvt,
                "bv": bv_,
                "mrow": mrows[p],
            }
        )
    return in_maps


def _get_program():
    if "nc" not in _CACHE:
        _CACHE["nc"] = _build_program()
    return _CACHE["nc"]


def run_on_device(in_maps, trace=False, trace_kwargs=None):
    from concourse import bass_utils

    nc = _get_program()
    return bass_utils.run_bass_kernel_spmd(
        nc,
        in_maps,
        core_ids=list(range(NCORES)),
        trace=trace,
        trace_kwargs=trace_kwargs or {},
    )


def kernel(x, Wq, bq, Wk, bk, Wv, bv):
    x = np.asarray(x, np.float32)
    in_maps = _host_inputs(x, Wq, bq, Wk, bk, Wv, bv)
    res = run_on_device(in_maps)
    att = np.empty((B, S, VD), np.float32)
    for b in range(B):
        att[b] = res.results[2 * b]["att"] + res.results[2 * b + 1]["att"]
    return np.concatenate([x, att], axis=2)


# revision 23
# speedup vs baseline: 1.0676x; 1.0676x over previous
"""Trainium2 Bass kernel for nn_AttentionBlock (column-softmax causal attention).

Reference computation (B=4, S=4096, D=128, K=64, V=128):
    Q = x @ Wq.T + bq            [B,S,64]
    Km = x @ Wk.T + bk           [B,S,64]
    Vm = x @ Wv.T + bv           [B,S,128]
    s  = Q @ Km.T / 8            [B,S,S], causal mask j>q -> -1e9
    p  = softmax(s, axis=1)      (softmax over the QUERY axis -- column softmax)
    att = p @ Vm                 [B,S,128]
    out = concat(x, att, dim=2)  [B,S,256]

Key observation: with ST = s.T (layout [j, q]) the softmax denominator
l[j] = sum_q exp(ST[j, q]) is a free-dim reduction, so
att[q] = sum_j exp(ST[j,q]) * (Vm[j]/l[j]) -- a flash-style two-phase kernel
with NO max subtraction needed (scores are O(+-20), exp is safe in fp32).

Sharding (8 cores): core c -> batch b = c//2, j-tile parity p = c%2.
Each core computes l[j] and the PV partial sum for its 16 j-tiles
(j-tile J = 2*i + p), over all q. Host adds the two partials per batch.
All parity differences are data-driven (xkv row gather + additive mask
input), so one SPMD program serves all 8 cores.

Performance structure per core:
  - QK score matmuls run as f32r with row-PAIR packing: rows 2r / 2r+1 use
    the two 64-partition halves of the PE array concurrently (KT/QT are
    duplicated into both partition halves so tile_position auto-derives).
  - exp runs on ACT with fused per-partition accumulation (accum_out = l).
  - PV runs transposed: attT[v, q] = sum_j Vp[j,v] * E[j,q] with N=512
    moving operands (bf16), then PE-transposes back to [q, v] tiles.
"""

import numpy as np

B, S, D = 4, 4096, 128
KD, VD = 64, 128
P = 128
NCORES = 8
JT = 16           # local j-tiles per core
NT = S // P       # 32 global q/j tiles
CHUNK = 1024      # ACT exp chunk width (PSUM cols)

QK_F32R = True

ROW_W = [S - 2 * i * P for i in range(JT)]          # E row widths
EOFF = [0] * JT
for _i in range(1, JT):
    EOFF[_i] = EOFF[_i - 1] + ROW_W[_i - 1]
ECOLS = EOFF[-1] + ROW_W[-1]                        # 34816

_CACHE = {}


def _build_program():
    from contextlib import ExitStack

    from concourse import bacc, mybir
    from concourse import tile as tile_mod

    dt = mybir.dt
    f32, bf16 = dt.float32, dt.bfloat16
    Alu = mybir.AluOpType
    ActF = mybir.ActivationFunctionType

    nc = bacc.Bacc(
        "TRN2", target_bir_lowering=False, debug=False, num_devices=NCORES
    )

    # Operand tiles of f32r matmuls must be PRODUCED as float32r (the BIR
    # verifier requires the producing instruction to round); they are
    # written by DVE ops (which round) or DMA'd in as float32r directly.
    mmdt = dt.float32r if QK_F32R else f32

    # Host supplies x^T / xkv^T / W^T (pure layout prep) so the kernel
    # spends no PE/ACT/DVE time transposing, and the DMAs are contiguous.
    xt_d = nc.dram_tensor("xt", [P, S], mmdt, kind="ExternalInput").ap()
    xkvt_d = nc.dram_tensor("xkvt", [P, JT * P], mmdt, kind="ExternalInput").ap()
    wqt_d = nc.dram_tensor("wqt", [P, 2 * KD], mmdt, kind="ExternalInput").ap()
    bq_d = nc.dram_tensor("bq", [P, 1], f32, kind="ExternalInput").ap()
    wkt_d = nc.dram_tensor("wkt", [P, 2 * KD], mmdt, kind="ExternalInput").ap()
    bk_d = nc.dram_tensor("bk", [P, 1], f32, kind="ExternalInput").ap()
    wvt_d = nc.dram_tensor("wvt", [P, VD], mmdt, kind="ExternalInput").ap()
    bv_d = nc.dram_tensor("bv", [VD, 1], f32, kind="ExternalInput").ap()
    mrow_d = nc.dram_tensor("mrow", [P, 2 * P], f32, kind="ExternalInput").ap()
    att_d = nc.dram_tensor("att", [S, VD], f32, kind="ExternalOutput").ap()

    with tile_mod.TileContext(nc) as tc, ExitStack() as ctx:
        persist = ctx.enter_context(tc.tile_pool(name="persist", bufs=1))

        xT = persist.tile([P, S], mmdt)            # [d, q]
        xkvT = persist.tile([P, JT * P], mmdt)     # [d, local j]
        # QT/KT live in BOTH partition halves (rows 0-63 == rows 64-127) so
        # QK row pairs can use tile_position (0,0)/(64,0) concurrently.
        QT = persist.tile([P, S], mmdt)            # [k(dup), q]
        KTl = persist.tile([P, JT * P], mmdt)      # [k(dup), local j]
        V_sb = persist.tile([P, JT, VD], f32)      # [local j, v]
        Vp_sb = persist.tile([P, JT, VD], bf16)    # V / l
        E_all = persist.tile([P, ECOLS], bf16)     # exp(scores.T) rows
        l_all = persist.tile([P, JT], f32)
        linv = persist.tile([P, JT], f32)
        WqT = persist.tile([P, 2 * KD], mmdt)      # Wq^T duplicated in free dim
        WkT = persist.tile([P, 2 * KD], mmdt)
        WvT = persist.tile([P, VD], mmdt)
        VT_sb = persist.tile([P, JT * P], f32)     # [v, local j]
        bq_sb = persist.tile([P, 1], f32)
        bk_sb = persist.tile([P, 1], f32)
        bv_sb = persist.tile([VD, 1], f32)
        mrow_sb = persist.tile([P, 2 * P], f32)
        ident = persist.tile([P, P], f32)

        # ---- input DMAs: tiny tensors first, then contiguous transposed
        # activations chunked so dependent compute starts early.
        nc.sync.dma_start(out=WqT, in_=wqt_d)
        nc.sync.dma_start(out=WkT, in_=wkt_d)
        nc.sync.dma_start(out=WvT, in_=wvt_d)
        nc.sync.dma_start(out=bq_sb, in_=bq_d)
        nc.sync.dma_start(out=bk_sb, in_=bk_d)
        nc.sync.dma_start(out=bv_sb, in_=bv_d)
        nc.sync.dma_start(out=mrow_sb, in_=mrow_d)
        for g in range(4):
            nc.sync.dma_start(
                out=xkvT[:, g * 512 : (g + 1) * 512],
                in_=xkvt_d[:, g * 512 : (g + 1) * 512],
            )
        for g in reversed(range(8)):
            nc.sync.dma_start(
                out=xT[:, g * 512 : (g + 1) * 512],
                in_=xt_d[:, g * 512 : (g + 1) * 512],
            )

        # identity for PE transposes
        nc.gpsimd.memset(ident, 0.0)
        nc.gpsimd.affine_select(
            out=ident,
            in_=ident,
            compare_op=Alu.not_equal,
            fill=1.0,
            base=0,
            pattern=[[-1, P]],
            channel_multiplier=1,
        )

        # ---- phase A/B: row pairs in REVERSE order, each preceded only by
        # the KT/QT projection chunks it needs (so the PE FIFO never blocks
        # on input DMA for data a later pair needs), with the V path spread
        # through the (ACT-bound) pair phase.
        with ExitStack() as pha:
            prj = pha.enter_context(
                tc.tile_pool(name="prj_psum", bufs=2, space="PSUM")
            )
            rowp = pha.enter_context(
                tc.tile_pool(name="row_psum", bufs=3, space="PSUM")
            )
            lpp = pha.enter_context(tc.tile_pool(name="lparts", bufs=8))

            def emit_kt_chunk(c):
                ps = prj.tile([P, 512], f32, tag="prj", name=f"kt_{c}")
                nc.tensor.matmul(
                    ps,
                    lhsT=WkT,
                    rhs=xkvT[:, c * 512 : (c + 1) * 512],
                    start=True,
                    stop=True,
                )
                nc.vector.tensor_scalar(
                    out=KTl[:, c * 512 : (c + 1) * 512],
                    in0=ps,
                    scalar1=bk_sb,
                    scalar2=None,
                    op0=Alu.add,
                )

            def emit_qt_chunk(c):
                ps = prj.tile([P, 512], f32, tag="prj", name=f"qt_{c}")
                nc.tensor.matmul(
                    ps,
                    lhsT=WqT,
                    rhs=xT[:, c * 512 : (c + 1) * 512],
                    start=True,
                    stop=True,
                )
                nc.vector.tensor_scalar(
                    out=QT[:, c * 512 : (c + 1) * 512],
                    in0=ps,
                    scalar1=bq_sb,
                    scalar2=None,
                    op0=Alu.add,
                )

            def emit_v_group(g):
                # VT chunk g -> V tiles [j, v] for rows 4g..4g+3, then
                # V' = V/l (their l is complete once pair 2g is done).
                ps = prj.tile([P, 512], f32, tag="prj", name=f"vt_{g}")
                nc.tensor.matmul(
                    ps,
                    lhsT=WvT,
                    rhs=xkvT[:, g * 512 : (g + 1) * 512],
                    start=True,
                    stop=True,
                )
                nc.vector.tensor_scalar(
                    out=VT_sb[:, g * 512 : (g + 1) * 512],
                    in0=ps,
                    scalar1=bv_sb,
                    scalar2=None,
                    op0=Alu.add,
                )
                pst = prj.tile([P, 4, P], f32, tag="prj", name=f"vtp_{g}")
                for k in range(4):
                    i = g * 4 + k
                    nc.tensor.transpose(
                        pst[:, k, :], VT_sb[:, i * P : (i + 1) * P], ident
                    )
                nc.vector.tensor_copy(
                    V_sb[:, g * 4 : (g + 1) * 4, :].rearrange(
                        "p a b -> p (a b)"
                    ),
                    pst.rearrange("p a b -> p (a b)"),
                )
                for i in range(4 * g, 4 * g + 4):
                    nc.vector.reciprocal(
                        linv[:, i : i + 1], l_all[:, i : i + 1]
                    )
                    nc.vector.tensor_scalar(
                        out=Vp_sb[:, i, :],
                        in0=V_sb[:, i, :],
                        scalar1=linv[:, i : i + 1],
                        scalar2=None,
                        op0=Alu.mult,
                    )

            def emit_row_pair(r):
                # rows 2r (partition half 0) and 2r+1 (half 64), MMs
                # interleaved at 512-slice granularity so the PE overlaps
                # them in opposite array halves.
                state = {}
                for i in (2 * r, 2 * r + 1):
                    q0 = 256 * i
                    w = ROW_W[i]
                    chunks = [
                        (q0 + c * CHUNK, min(CHUNK, w - c * CHUNK))
                        for c in range((w + CHUNK - 1) // CHUNK)
                    ]
                    slices = []
                    for ci, (off, cw) in enumerate(chunks):
                        for s0 in range(0, cw, 512):
                            slices.append((ci, off, cw, s0, min(512, cw - s0)))
                    state[i] = {"chunks": chunks, "slices": slices, "ps": {}}

                def finish_chunk(i, ci, cw):
                    st = state[i]
                    ps = st["ps"][ci]
                    if ci == 0:
                        nc.vector.tensor_add(
                            ps[:, : 2 * P], ps[:, : 2 * P], mrow_sb
                        )
                    lp = lpp.tile([P, 1], f32, tag="lp", name=f"lp_{i}_{ci}")
                    ecol = EOFF[i] + ci * CHUNK
                    nc.scalar.activation(
                        out=E_all[:, ecol : ecol + cw],
                        in_=ps[:, :cw],
                        func=ActF.Exp,
                        accum_out=lp,
                    )
                    if ci == 0:
                        nc.vector.tensor_copy(l_all[:, i : i + 1], lp)
                    else:
                        nc.vector.tensor_add(
                            l_all[:, i : i + 1], l_all[:, i : i + 1], lp
                        )

                nslice = max(len(state[i]["slices"]) for i in state)
                for k in range(nslice):
                    for idx, i in enumerate((2 * r, 2 * r + 1)):
                        st = state[i]
                        if k >= len(st["slices"]):
                            continue
                        ci, off, cw, s0, sw = st["slices"][k]
                        if ci not in st["ps"]:
                            st["ps"][ci] = rowp.tile(
                                [P, CHUNK], f32, tag="st", name=f"st_{i}_{ci}"
                            )
                        base = KD * idx
                        nc.tensor.matmul(
                            st["ps"][ci][:, s0 : s0 + sw],
                            lhsT=KTl[base : base + KD, i * P : (i + 1) * P],
                            rhs=QT[base : base + KD, off + s0 : off + s0 + sw],
                            start=True,
                            stop=True,
                        )
                        if s0 + sw == cw:
                            finish_chunk(i, ci, cw)

            kt_done = set()
            for r in reversed(range(8)):
                if r // 2 not in kt_done:
                    kt_done.add(r // 2)
                    emit_kt_chunk(r // 2)
                emit_qt_chunk(r)
                emit_row_pair(r)
                if r % 2 == 0:
                    emit_v_group(r // 2)

        # ---- phase C: PV block ------------------------------------------
        with ExitStack() as phc:
            attp = phc.enter_context(
                tc.tile_pool(name="att_psum", bufs=4, space="PSUM")
            )
            tsbp = phc.enter_context(tc.tile_pool(name="attT_sb", bufs=2))
            sbo = phc.enter_context(tc.tile_pool(name="att_sb", bufs=2))

            def emit_pv_chunk(c):
                # attT[v, q] for q in [512c, 512c+512): rows i <= 2c full
                # coverage (N=512), row 2c+1 covers the second half (N=256).
                attT = attp.tile([P, 4, P], f32, tag="attT", name=f"attT_{c}")
                aflat = attT.rearrange("p a b -> p (a b)")
                for i in range(2 * c + 1):
                    ecol = EOFF[i] + 512 * c - 256 * i
                    nc.tensor.matmul(
                        aflat,
                        lhsT=Vp_sb[:, i, :],
                        rhs=E_all[:, ecol : ecol + 512],
                        start=(i == 0),
                        stop=False,
                    )
                i2 = 2 * c + 1
                nc.tensor.matmul(
                    aflat[:, 256:512],
                    lhsT=Vp_sb[:, i2, :],
                    rhs=E_all[:, EOFF[i2] : EOFF[i2] + 256],
                    start=False,
                    stop=True,
                )
                tsb = tsbp.tile([P, 4, P], f32, tag="tsb", name=f"tsb_{c}")
                nc.scalar.copy(tsb.rearrange("p a b -> p (a b)"), aflat)
                outq = attp.tile([P, 4, P], f32, tag="attT", name=f"outq_{c}")
                for k in range(4):
                    nc.tensor.transpose(outq[:, k, :], tsb[:, k, :], ident)
                osb = sbo.tile([P, 4, P], f32, tag="osb", name=f"osb_{c}")
                nc.vector.tensor_copy(
                    osb.rearrange("p a b -> p (a b)"),
                    outq.rearrange("p a b -> p (a b)"),
                )
                nc.sync.dma_start(
                    out=att_d[c * 512 : (c + 1) * 512, :].rearrange(
                        "(t p) v -> p t v", p=P
                    ),
                    in_=osb,
                )

            for c in reversed(range(8)):
                emit_pv_chunk(c)

    nc.compile()
    return nc


def _host_inputs(x, Wq, bq, Wk, bk, Wv, bv):
    """Per-core input maps (host does layout prep: transposes + gathers)."""
    x_full = np.ascontiguousarray(x, dtype=np.float32)
    Wq_s = np.asarray(Wq, np.float32) / 8.0
    wqt = np.ascontiguousarray(np.concatenate([Wq_s.T, Wq_s.T], axis=1))
    bq_s = np.tile((np.asarray(bq, np.float32) / 8.0).reshape(KD, 1), (2, 1))
    WkT_ = np.asarray(Wk, np.float32).T
    wkt = np.ascontiguousarray(np.concatenate([WkT_, WkT_], axis=1))
    bk_ = np.tile(np.asarray(bk, np.float32).reshape(KD, 1), (2, 1))
    wvt = np.ascontiguousarray(np.asarray(Wv, np.float32).T)
    bv_ = np.asarray(bv, np.float32).reshape(VD, 1)

    tri = np.where(
        np.arange(P)[None, :] >= np.arange(P)[:, None], 0.0, -1e9
    ).astype(np.float32)
    mrows = []
    for p in (0, 1):
        m = np.zeros((P, 2 * P), np.float32)
        if p == 0:
            m[:, :P] = tri
        else:
            m[:, :P] = -1e9
            m[:, P:] = tri
        mrows.append(m)

    in_maps = []
    xts = [np.ascontiguousarray(x_full[b].T) for b in range(B)]
    for c in range(NCORES):
        b, p = c // 2, c % 2
        xkvt = np.ascontiguousarray(
            x_full[b].reshape(NT, P, D)[p::2].reshape(JT * P, D).T
        )
        in_maps.append(
            {
                "xt": xts[b],
                "xkvt": xkvt,
                "wqt": wqt,
                "bq": bq_s,
                "wkt": wkt,
                "bk": bk_,
                "wvt": wvt,
                "bv": bv_,
                "mrow": mrows[p],
            }
        )
    return in_maps


def _get_program():
    if "nc" not in _CACHE:
        _CACHE["nc"] = _build_program()
    return _CACHE["nc"]


def run_on_device(in_maps, trace=False, trace_kwargs=None):
    from concourse import bass_utils

    nc = _get_program()
    return bass_utils.run_bass_kernel_spmd(
        nc,
        in_maps,
        core_ids=list(range(NCORES)),
        trace=trace,
        trace_kwargs=trace_kwargs or {},
    )


def kernel(x, Wq, bq, Wk, bk, Wv, bv):
    x = np.asarray(x, np.float32)
    in_maps = _host_inputs(x, Wq, bq, Wk, bk, Wv, bv)
    res = run_on_device(in_maps)
    att = np.empty((B, S, VD), np.float32)
    for b in range(B):
        att[b] = res.results[2 * b]["att"] + res.results[2 * b + 1]["att"]
    return np.concatenate([x, att], axis=2)
